# revision 7
# baseline (speedup 1.0000x reference)
"""Trainium2 Bass kernel for nn_AttnGate (block-sparse attention gate).

Computation (per batch b, kv-head kh):
    qp  = einsum('s(gd),(gd)o->so', q[b,:,4kh:4kh+4,:], wq[kh])       # [S, GH]
    qpR = rope(qp, cos_q[b], sin_q[b])
    kc  = [mean, max] pooling of k[b,:,kh,:] over 64-key blocks        # [NB, 2D]
    kp  = kc @ wk[kh];  kpR = rope(kp, cos_k[b], sin_k[b])             # [NB, GH]
    out = softmax(qpR @ kpR.T / sqrt(GH) + mask[b])                    # [S, NB]

Sharding: 16 (b, kh) units over 8 cores; core c handles b = c//4 and
kh in {2*(c%4), 2*(c%4)+1}.  No collectives; host slices/gathers.

Host-side layout prep (inside kernel(), numpy only):
  - q/k slices pre-transposed to contraction-major ([gd, S] / [d, S]) so no
    on-chip fp32 transposes are needed.
  - 1/sqrt(GH) folded into wq; 1/64 mean-pool scale folded into wk[:, :D].
  - RoPE rotate-half is folded into the QK matmul:
        attn[s,n] = sum_h qp[s,h]*cos[s,h]*kpR[n,h]
                  + sum_h qp[s,h]*tq[s,h]*kq2[n,h]
    with tq = swap_halves(sin_q) (upper half negated) and
    kq2 = swap_halves(kpR) (computed on-chip via a permutation matmul), so
    the per-s-row work is just two elementwise multiplies.
  - mask is pre-cast to bf16 (exact for 0/-1e9) and added into PSUM with an
    identity-matrix matmul.
"""

import math
import os
import sys

import numpy as np

for _p in ("/opt/trn_rl_repo", "/root/.axon_site/_ro/trn_rl_repo"):
    if os.path.isdir(_p) and _p not in sys.path:
        sys.path.append(_p)

import ml_dtypes  # noqa: E402

import concourse.bass as bass  # noqa: E402
import concourse.bacc as bacc  # noqa: E402
import concourse.mybir as mybir  # noqa: E402
from concourse.bass_utils import run_bass_kernel_spmd  # noqa: E402
from concourse.tile import TileContext  # noqa: E402

# Problem dims (hardcoded per spec).
B, S, HQ, HK, D, GH = 2, 8192, 32, 8, 128, 128
BLK = 64
NB = S // BLK          # 128 key blocks
G = HQ // HK           # 4
GD = G * D             # 512 contraction for the q projection
H = GH // 2            # rotate-half split
ST = 512               # s-tile (matmul moving-dim)
NT = S // ST           # 16 s-tiles per unit
CPT = ST // 128        # 128-row chunks per s-tile (4)
N_CORES = 8

F32 = mybir.dt.float32
F32R = mybir.dt.float32r
BF16 = mybir.dt.bfloat16
FP = mybir.dt.np  # dt -> numpy dtype

UNITS = int(os.environ.get('KERNEL_UNITS', '2'))

# Dtype knobs (precision/perf tradeoffs; f32 everywhere except the mask path).
QPROJ_DT = F32R       # q-projection matmul mode (f32r: full PE rate at N=512)
QK_DT = F32           # dtype of a/bq/kpR operands of the score matmul
COS_DT = F32          # resident cos/tq tables


def _np_dt(dt):
    return np.dtype(FP(dt))


def _cast(ap, dt):
    return ap if ap.dtype == dt else ap.bitcast(dt)


def build_bass():
    """Build the single-core SPMD Bass program (same NEFF on all 8 cores)."""
    nc = bacc.Bacc("TRN2")

    qT = nc.declare_dram_parameter("qT", [2, G, D, S], QPROJ_DT, isOutput=False)
    kT = nc.declare_dram_parameter("kT", [2, D, S], F32, isOutput=False)
    wq = nc.declare_dram_parameter("wq", [2, G, D, GH], QPROJ_DT, isOutput=False)
    wk = nc.declare_dram_parameter("wk", [2, 2, D, GH], F32, isOutput=False)
    maskp = nc.declare_dram_parameter("maskp", [S, NB], BF16, isOutput=False)
    cosq = nc.declare_dram_parameter("cosq", [GH, S], COS_DT, isOutput=False)
    tq = nc.declare_dram_parameter("tq", [GH, S], COS_DT, isOutput=False)
    cosk = nc.declare_dram_parameter("cosk", [GH, NB], F32, isOutput=False)
    sinks = nc.declare_dram_parameter("sinks", [GH, NB], F32, isOutput=False)
    ident = nc.declare_dram_parameter("ident", [128, 128], BF16, isOutput=False)
    pswap = nc.declare_dram_parameter("pswap", [128, 128], F32, isOutput=False)
    out = nc.declare_dram_parameter("out", [2, S, NB], F32, isOutput=True)

    NQUART = 4
    SQ = S // NQUART  # 2048 cols per resident-table quarter

    with TileContext(nc) as tc:
        with (
            tc.tile_pool(name="singles", bufs=1) as singles,
            tc.tile_pool(name="ktp", bufs=1) as ktp,
            tc.tile_pool(name="kstat", bufs=2) as kstat,
            tc.tile_pool(name="krope", bufs=2) as krope,
            tc.tile_pool(name="qin", bufs=3) as qin,
            tc.tile_pool(name="qps", bufs=2, space="PSUM") as qps,
            tc.tile_pool(name="kpps", bufs=2, space="PSUM") as kpps,
            tc.tile_pool(name="atps", bufs=4, space="PSUM") as atps,
            tc.tile_pool(name="qpsb", bufs=2) as qpsb,
            tc.tile_pool(name="absb", bufs=2) as absb,
            tc.tile_pool(name="esb", bufs=8) as esb,
            tc.tile_pool(name="ssb", bufs=2) as ssb,
            tc.tile_pool(name="osb", bufs=3) as osb,
        ):
            # ---- resident constants ----
            wq_sb = {}
            for j in range(2):
                for c in range(G):
                    t = singles.tile([D, GH], QPROJ_DT, name=f"wq_{j}_{c}", tag=f"wq{j}{c}")
                    nc.sync.dma_start(out=t, in_=wq[j, c])
                    wq_sb[j, c] = t
            wk_sb = {}
            for j in range(2):
                for c in range(2):
                    t = singles.tile([D, GH], F32, name=f"wk_{j}_{c}", tag=f"wk{j}{c}")
                    nc.sync.dma_start(out=t, in_=wk[j, c])
                    wk_sb[j, c] = t
            cosk_sb = singles.tile([GH, NB], F32, name="cosk_sb", tag="cosk")
            nc.sync.dma_start(out=cosk_sb, in_=cosk[:, :])
            sinks_sb = singles.tile([GH, NB], F32, name="sinks_sb", tag="sinks")
            nc.sync.dma_start(out=sinks_sb, in_=sinks[:, :])
            ident_sb = singles.tile([128, 128], BF16, name="ident_sb", tag="ident")
            nc.sync.dma_start(out=ident_sb, in_=ident[:, :])
            pswap_sb = singles.tile([128, 128], F32, name="pswap_sb", tag="pswap")
            nc.sync.dma_start(out=pswap_sb, in_=pswap[:, :])

            # Resident tables, split into quarters so early tiles start sooner.
            cos_sb, tq_sb, mask_sb = [], [], []
            for qi in range(NQUART):
                t = singles.tile([GH, SQ], COS_DT, name=f"cos_sb_{qi}", tag=f"cosq{qi}")
                nc.sync.dma_start(out=t, in_=cosq[:, qi * SQ:(qi + 1) * SQ])
                cos_sb.append(t)
                t = singles.tile([GH, SQ], COS_DT, name=f"tq_sb_{qi}", tag=f"tqq{qi}")
                nc.sync.dma_start(out=t, in_=tq[:, qi * SQ:(qi + 1) * SQ])
                tq_sb.append(t)
                t = singles.tile(
                    [128, SQ // 128, NB], BF16, name=f"mask_sb_{qi}", tag=f"mask{qi}"
                )
                nc.sync.dma_start(
                    out=t,
                    in_=maskp[qi * SQ:(qi + 1) * SQ, :].rearrange(
                        "(c p) n -> p c n", p=128
                    ),
                )
                mask_sb.append(t)

            for j in range(UNITS):  # the (b, kh) units on this core
                # ---- K path ----
                kt_sb = ktp.tile([D, S], F32, name=f"kt_{j}", tag="kt")
                nc.sync.dma_start(out=kt_sb, in_=kT[j])
                kt3 = kt_sb.rearrange("p (n b) -> p n b", b=BLK)
                ksum = kstat.tile([D, NB], F32, name=f"ksum_{j}", tag="ksum")
                nc.vector.reduce_sum(out=ksum, in_=kt3, axis=mybir.AxisListType.X)
                kmax = kstat.tile([D, NB], F32, name=f"kmax_{j}", tag="kmax")
                nc.vector.reduce_max(out=kmax, in_=kt3, axis=mybir.AxisListType.X)

                kp_ps = kpps.tile([GH, NB], F32, name=f"kp_ps_{j}", tag="kp")
                nc.tensor.matmul(
                    kp_ps, wk_sb[j, 0], ksum,
                    start=True, stop=False,
                )
                nc.tensor.matmul(
                    kp_ps, wk_sb[j, 1], kmax,
                    start=False, stop=True,
                )
                kp_sb = krope.tile([GH, NB], F32, name=f"kp_sb_{j}", tag="kpsb")
                nc.scalar.copy(kp_sb, kp_ps)
                # kp2 = swap_halves(kp) via permutation matmul
                kp2_ps = kpps.tile([GH, NB], F32, name=f"kp2_ps_{j}", tag="kp")
                nc.tensor.matmul(
                    kp2_ps, pswap_sb, kp_sb,
                    start=True, stop=True,
                )
                # kpR = kp*cosk + kp2*sinks  (sign folded into sinks on host)
                u_sb = krope.tile([GH, NB], F32, name=f"u_sb_{j}", tag="usb")
                nc.vector.tensor_mul(u_sb, kp_sb, cosk_sb)
                v_sb = krope.tile([GH, NB], F32, name=f"v_sb_{j}", tag="vsb")
                nc.vector.tensor_mul(v_sb, kp2_ps, sinks_sb)
                kpr_sb = krope.tile([GH, NB], QK_DT, name=f"kpr_sb_{j}", tag="kpr")
                nc.vector.tensor_add(kpr_sb, u_sb, v_sb)
                # kq2 = swap_halves(kpR) via permutation matmul
                kq2_ps = kpps.tile([GH, NB], F32, name=f"kq2_ps_{j}", tag="kp")
                nc.tensor.matmul(
                    kq2_ps, pswap_sb, kpr_sb,
                    start=True, stop=True,
                )
                kq2_sb = krope.tile([GH, NB], QK_DT, name=f"kq2_sb_{j}", tag="kq2")
                nc.scalar.copy(kq2_sb, kq2_ps)

                # ---- Q path + scores + softmax, streamed over s ----
                for it in range(NT):
                    s0 = it * ST
                    qi = s0 // SQ
                    sq0 = s0 - qi * SQ

                    q_sb = qin.tile([D, G, ST], QPROJ_DT, name=f"q_{j}_{it}", tag="q")
                    nc.sync.dma_start(
                        out=q_sb,
                        in_=qT[j, :, :, s0:s0 + ST].rearrange("c p s -> p c s"),
                    )
                    qp_ps = qps.tile([GH, ST], F32, name=f"qp_{j}_{it}", tag="qp")
                    for c in range(G):
                        nc.tensor.matmul(
                            qp_ps,
                            wq_sb[j, c],
                            q_sb[:, c, :],
                            start=(c == 0),
                            stop=(c == G - 1),
                        )
                    qp_sb = qpsb.tile([GH, ST], F32, name=f"qpsb_{j}_{it}", tag="qpsb")
                    nc.scalar.copy(qp_sb, qp_ps)
                    a_sb = absb.tile([GH, ST], QK_DT, name=f"a_{j}_{it}", tag="a")
                    nc.vector.tensor_mul(a_sb, qp_sb, cos_sb[qi][:, sq0:sq0 + ST])
                    b_sb = absb.tile([GH, ST], QK_DT, name=f"b_{j}_{it}", tag="b")
                    nc.vector.tensor_mul(b_sb, qp_sb, tq_sb[qi][:, sq0:sq0 + ST])

                    sums = ssb.tile([128, CPT], F32, name=f"sums_{j}_{it}", tag="sums")
                    rec = ssb.tile([128, CPT], F32, name=f"rec_{j}_{it}", tag="rec")
                    o_sb = osb.tile([128, CPT, NB], F32, name=f"o_{j}_{it}", tag="o")
                    e_cc = []
                    for cc in range(CPT):
                        col = slice(cc * 128, cc * 128 + 128)
                        at_ps = atps.tile(
                            [128, NB], F32, name=f"at_{j}_{it}_{cc}", tag="at"
                        )
                        nc.tensor.matmul(
                            at_ps, a_sb[:, col], kpr_sb,
                            start=True, stop=False,
                        )
                        nc.tensor.matmul(
                            at_ps, b_sb[:, col], kq2_sb,
                            start=False, stop=False,
                        )
                        gc = it * CPT + cc
                        nc.tensor.matmul(
                            at_ps, ident_sb, mask_sb[qi][:, gc - qi * (SQ // 128), :],
                            start=False, stop=True,
                        )
                        e_sb = esb.tile([128, NB], F32, name=f"e_{j}_{it}_{cc}", tag="e")
                        nc.scalar.activation(
                            e_sb, at_ps, mybir.ActivationFunctionType.Exp,
                            accum_out=sums[:, cc:cc + 1],
                        )
                        e_cc.append(e_sb)
                    nc.vector.reciprocal(rec, sums)
                    for cc in range(CPT):
                        nc.vector.tensor_scalar_mul(
                            o_sb[:, cc, :], e_cc[cc], rec[:, cc:cc + 1]
                        )
                    nc.sync.dma_start(
                        out=out[j, s0:s0 + ST, :].rearrange("(c p) n -> p c n", p=128),
                        in_=o_sb,
                    )
    nc.compile()
    return nc


_BUILT = None


def _get_built():
    global _BUILT
    if _BUILT is None:
        _BUILT = build_bass()
    return _BUILT


def prep_inputs(q, k, attention_mask, cos_q, sin_q, cos_k, sin_k, wq, wk):
    """Slice + lay out the full inputs into 8 per-core input maps (numpy)."""
    q = np.asarray(q, dtype=np.float32)
    k = np.asarray(k, dtype=np.float32)
    attention_mask = np.asarray(attention_mask, dtype=np.float32)
    cos_q = np.asarray(cos_q, dtype=np.float32)
    sin_q = np.asarray(sin_q, dtype=np.float32)
    cos_k = np.asarray(cos_k, dtype=np.float32)
    sin_k = np.asarray(sin_k, dtype=np.float32)
    wq = np.asarray(wq, dtype=np.float32)
    wk = np.asarray(wk, dtype=np.float32)

    scale = np.float32(1.0 / math.sqrt(GH))
    wq_s = (wq * scale).astype(np.float32)                  # fold score scale
    wk_f = wk.reshape(HK, 2, D, GH).copy()
    wk_f[:, 0, :, :] *= np.float32(1.0 / BLK)               # fold mean pooling

    ident = np.eye(128, dtype=_np_dt(BF16))
    pswap = np.zeros((128, 128), dtype=np.float32)
    pswap[(np.arange(128) + H) % 128, np.arange(128)] = 1.0

    cos_dt = _np_dt(COS_DT)
    in_maps = []
    for core in range(N_CORES):
        b = core // 4
        kh0 = 2 * (core % 4)
        # qT: [2, G, D, S]
        qs = q[b, :, 4 * kh0:4 * kh0 + 8, :]                # [S, 8, D]
        qT = np.ascontiguousarray(
            qs.reshape(S, 2, G, D).transpose(1, 2, 3, 0)
        )
        kT = np.ascontiguousarray(
            k[b, :, kh0:kh0 + 2, :].transpose(1, 2, 0)      # [2, D, S]
        )
        cq = cos_q[b]                                       # [S, GH]
        sq = sin_q[b]
        # tq[h, s] = sin[s, h+64] for h<64 ; -sin[s, h-64] for h>=64
        tq = np.concatenate([sq[:, H:], -sq[:, :H]], axis=1).T
        ck = cos_k[b]                                       # [NB, GH]
        sk = sin_k[b]
        # sinks[g, n] = -sin_k[n, g] for g<64 ; +sin_k[n, g-?]... sign on lower
        sks = np.concatenate([-sk[:, :H], sk[:, H:]], axis=1).T
        in_maps.append({
            "qT": qT,
            "kT": kT,
            "wq": np.ascontiguousarray(wq_s[kh0:kh0 + 2].reshape(2, G, D, GH)),
            "wk": np.ascontiguousarray(wk_f[kh0:kh0 + 2]),
            "maskp": attention_mask[b, 0].astype(_np_dt(BF16)),
            "cosq": np.ascontiguousarray(cq.T).astype(cos_dt),
            "tq": np.ascontiguousarray(tq).astype(cos_dt),
            "cosk": np.ascontiguousarray(ck.T),
            "sinks": np.ascontiguousarray(sks),
            "ident": ident,
            "pswap": pswap,
        })
    return in_maps


def run_cores(in_maps, **kwargs):
    nc = _get_built()
    return run_bass_kernel_spmd(nc, in_maps, core_ids=list(range(N_CORES)), **kwargs)


def kernel(**inputs):
    in_maps = prep_inputs(**inputs)
    res = run_cores(in_maps)
    full = np.empty((B, HK, S, NB), dtype=np.float32)
    for core in range(N_CORES):
        b = core // 4
        kh0 = 2 * (core % 4)
        o = np.asarray(res.results[core]["out"], dtype=np.float32)
        full[b, kh0] = o[0]
        full[b, kh0 + 1] = o[1]
    return full


# revision 8
# speedup vs baseline: 1.5566x; 1.5566x over previous
"""Trainium2 Bass kernel for nn_AttnGate (block-sparse attention gate).

Computation (per batch b, kv-head kh):
    qp  = einsum('s(gd),(gd)o->so', q[b,:,4kh:4kh+4,:], wq[kh])       # [S, GH]
    qpR = rope(qp, cos_q[b], sin_q[b])
    kc  = [mean, max] pooling of k[b,:,kh,:] over 64-key blocks        # [NB, 2D]
    kp  = kc @ wk[kh];  kpR = rope(kp, cos_k[b], sin_k[b])             # [NB, GH]
    out = softmax(qpR @ kpR.T / sqrt(GH) + mask[b])                    # [S, NB]

Sharding: 16 (b, kh) units over 8 cores; core c handles b = c//4 and
kh in {2*(c%4), 2*(c%4)+1}.  No collectives; host slices/gathers.

Host-side layout prep (inside kernel(), numpy only):
  - q/k slices pre-transposed to contraction-major ([gd, S] / [d, S]) so no
    on-chip fp32 transposes are needed; q-path and score operands pre-cast to
    fp16 (|values| ~ O(1), well inside fp16 range).
  - 1/sqrt(GH) folded into wq; 1/64 mean-pool scale folded into wk[:, :D].
  - RoPE rotate-half is folded into the QK matmul:
        attn[s,n] = sum_h qp[s,h]*cos[s,h]*kpR[n,h]
                  + sum_h qp[s,h]*tq[s,h]*kq2[n,h]
    with tq = swap_halves(sin_q) (upper half negated) and
    kq2 = swap_halves(kpR) (computed on-chip via a permutation matmul), so
    the per-s-row work is just two elementwise multiplies.
  - mask is pre-cast to bf16 (exact for 0/-1e9) and added into PSUM with an
    identity-matrix matmul.
  - block pooling is a pairwise fp16 tensor-tensor tree (2x DVE mode) rather
    than a 1x tensor_reduce.
"""

import math
import os
import sys

import numpy as np

for _p in ("/opt/trn_rl_repo", "/root/.axon_site/_ro/trn_rl_repo"):
    if os.path.isdir(_p) and _p not in sys.path:
        sys.path.append(_p)

import ml_dtypes  # noqa: E402,F401

import concourse.bass as bass  # noqa: E402,F401
import concourse.bacc as bacc  # noqa: E402
import concourse.mybir as mybir  # noqa: E402
from concourse.bass_utils import run_bass_kernel_spmd  # noqa: E402
from concourse.tile import TileContext  # noqa: E402

# Problem dims (hardcoded per spec).
B, S, HQ, HK, D, GH = 2, 8192, 32, 8, 128, 128
BLK = 64
NB = S // BLK          # 128 key blocks
G = HQ // HK           # 4
GD = G * D             # 512 contraction for the q projection
H = GH // 2            # rotate-half split
ST = 512               # s-tile (matmul moving-dim)
NT = S // ST           # 16 s-tiles per unit
CPT = ST // 128        # 128-row chunks per s-tile (4)
N_CORES = 8

F32 = mybir.dt.float32
F32R = mybir.dt.float32r
BF16 = mybir.dt.bfloat16
FP16 = mybir.dt.float16
FP = mybir.dt.np  # dt -> numpy dtype

UNITS = int(os.environ.get("KERNEL_UNITS", "2"))

# Dtype knobs (precision/perf tradeoffs).
QPROJ_DT = FP16       # q, wq and the q-projection matmul
QK_DT = FP16          # qp/a/bq/kpr/kq2 and the score matmuls
COS_DT = FP16         # resident cos/tq tables
KT_DT = FP16          # k in [d, s] layout + pooling tree
NORM_ON_ACT = True    # softmax normalize multiply on ScalarE (else VectorE)


def _np_dt(dt):
    return np.dtype(FP(dt))


def build_bass():
    """Build the single-core SPMD Bass program (same NEFF on all 8 cores)."""
    nc = bacc.Bacc("TRN2")

    qT = nc.declare_dram_parameter("qT", [2, G, D, S], QPROJ_DT, isOutput=False)
    kT = nc.declare_dram_parameter("kT", [2, D, S], KT_DT, isOutput=False)
    wq = nc.declare_dram_parameter("wq", [2, G, D, GH], QPROJ_DT, isOutput=False)
    wk = nc.declare_dram_parameter("wk", [2, 2, D, GH], F32, isOutput=False)
    maskp = nc.declare_dram_parameter("maskp", [S, NB], BF16, isOutput=False)
    cosq = nc.declare_dram_parameter("cosq", [GH, S], COS_DT, isOutput=False)
    tq = nc.declare_dram_parameter("tq", [GH, S], COS_DT, isOutput=False)
    cosk = nc.declare_dram_parameter("cosk", [GH, NB], F32, isOutput=False)
    sinks = nc.declare_dram_parameter("sinks", [GH, NB], F32, isOutput=False)
    ident = nc.declare_dram_parameter("ident", [128, 128], BF16, isOutput=False)
    pswap = nc.declare_dram_parameter("pswap", [128, 128], QK_DT, isOutput=False)
    out = nc.declare_dram_parameter("out", [2, S, NB], F32, isOutput=True)

    NQUART = 4
    SQ = S // NQUART  # 2048 cols per resident-table quarter

    with TileContext(nc) as tc:
        with (
            tc.tile_pool(name="singles", bufs=1) as singles,
            tc.tile_pool(name="ktp", bufs=2) as ktp,
            tc.tile_pool(name="pooltmp", bufs=2) as pooltmp,
            tc.tile_pool(name="kstat", bufs=2) as kstat,
            tc.tile_pool(name="krope", bufs=2) as krope,
            tc.tile_pool(name="qin", bufs=4) as qin,
            tc.tile_pool(name="qps", bufs=2, space="PSUM") as qps,
            tc.tile_pool(name="kpps", bufs=2, space="PSUM") as kpps,
            tc.tile_pool(name="atps", bufs=3, space="PSUM") as atps,
            tc.tile_pool(name="qpsb", bufs=3) as qpsb,
            tc.tile_pool(name="absb", bufs=3) as absb,
            tc.tile_pool(name="esb", bufs=3) as esb,
            tc.tile_pool(name="ssb", bufs=3) as ssb,
            tc.tile_pool(name="osb", bufs=3) as osb,
        ):
            # ---- resident constants ----
            wq_sb = {}
            for j in range(UNITS):
                for c in range(G):
                    t = singles.tile(
                        [D, GH], QPROJ_DT, name=f"wq_{j}_{c}", tag=f"wq{j}{c}"
                    )
                    nc.sync.dma_start(out=t, in_=wq[j, c])
                    wq_sb[j, c] = t
            wk_sb = {}
            for j in range(UNITS):
                for c in range(2):
                    t = singles.tile([D, GH], F32, name=f"wk_{j}_{c}", tag=f"wk{j}{c}")
                    nc.sync.dma_start(out=t, in_=wk[j, c])
                    wk_sb[j, c] = t
            cosk_sb = singles.tile([GH, NB], F32, name="cosk_sb", tag="cosk")
            nc.sync.dma_start(out=cosk_sb, in_=cosk[:, :])
            sinks_sb = singles.tile([GH, NB], F32, name="sinks_sb", tag="sinks")
            nc.sync.dma_start(out=sinks_sb, in_=sinks[:, :])
            ident_sb = singles.tile([128, 128], BF16, name="ident_sb", tag="ident")
            nc.sync.dma_start(out=ident_sb, in_=ident[:, :])
            pswap_sb = singles.tile([128, 128], QK_DT, name="pswap_sb", tag="pswap")
            nc.sync.dma_start(out=pswap_sb, in_=pswap[:, :])

            # Resident tables, split into quarters so early tiles start sooner.
            cos_sb, tq_sb, mask_sb = [], [], []
            for qi in range(NQUART):
                t = singles.tile([GH, SQ], COS_DT, name=f"cos_sb_{qi}", tag=f"cosq{qi}")
                nc.sync.dma_start(out=t, in_=cosq[:, qi * SQ:(qi + 1) * SQ])
                cos_sb.append(t)
                t = singles.tile([GH, SQ], COS_DT, name=f"tq_sb_{qi}", tag=f"tqq{qi}")
                nc.sync.dma_start(out=t, in_=tq[:, qi * SQ:(qi + 1) * SQ])
                tq_sb.append(t)
                t = singles.tile(
                    [128, SQ // 128, NB], BF16, name=f"mask_sb_{qi}", tag=f"mask{qi}"
                )
                nc.sync.dma_start(
                    out=t,
                    in_=maskp[qi * SQ:(qi + 1) * SQ, :].rearrange(
                        "(c p) n -> p c n", p=128
                    ),
                )
                mask_sb.append(t)

            def pool_tree(kt_sb, op, outname):
                """Pairwise-[op] over the 64-key blocks of kt_sb [D, S] ->
                [D, NB] f32, via fp16 2x tensor_tensor levels."""
                cur = kt_sb.rearrange("p (n b) -> p n b", b=BLK)
                width = BLK
                while width > 2:
                    width //= 2
                    t = pooltmp.tile(
                        [D, NB, width], KT_DT,
                        name=f"{outname}_l{width}", tag=f"pool{width}",
                    )
                    nc.vector.tensor_tensor(
                        out=t, in0=cur[:, :, :width], in1=cur[:, :, width:], op=op
                    )
                    cur = t
                res = kstat.tile([D, NB], F32, name=outname, tag=outname[:4])
                nc.vector.tensor_tensor(
                    out=res, in0=cur[:, :, 0], in1=cur[:, :, 1], op=op
                )
                return res

            for j in range(UNITS):  # the (b, kh) units on this core
                # ---- K path ----
                kt_sb = ktp.tile([D, S], KT_DT, name=f"kt_{j}", tag="kt")
                nc.sync.dma_start(out=kt_sb, in_=kT[j])
                ksum = pool_tree(kt_sb, mybir.AluOpType.add, f"ksum_{j}")
                kmax = pool_tree(kt_sb, mybir.AluOpType.max, f"kmax_{j}")

                kp_ps = kpps.tile([GH, NB], F32, name=f"kp_ps_{j}", tag="kp")
                nc.tensor.matmul(kp_ps, wk_sb[j, 0], ksum, start=True, stop=False)
                nc.tensor.matmul(kp_ps, wk_sb[j, 1], kmax, start=False, stop=True)
                kp_sb = krope.tile([GH, NB], QK_DT, name=f"kp_sb_{j}", tag="kpsb")
                nc.scalar.copy(kp_sb, kp_ps)
                # kp2 = swap_halves(kp) via permutation matmul
                kp2_ps = kpps.tile([GH, NB], F32, name=f"kp2_ps_{j}", tag="kp")
                nc.tensor.matmul(kp2_ps, pswap_sb, kp_sb, start=True, stop=True)
                # kpR = kp*cosk + kp2*sinks  (sign folded into sinks on host)
                u_sb = krope.tile([GH, NB], F32, name=f"u_sb_{j}", tag="usb")
                nc.vector.tensor_mul(u_sb, kp_ps, cosk_sb)
                v_sb = krope.tile([GH, NB], F32, name=f"v_sb_{j}", tag="vsb")
                nc.vector.tensor_mul(v_sb, kp2_ps, sinks_sb)
                kpr_sb = krope.tile([GH, NB], QK_DT, name=f"kpr_sb_{j}", tag="kpr")
                nc.vector.tensor_add(kpr_sb, u_sb, v_sb)
                # kq2 = swap_halves(kpR) via permutation matmul
                kq2_ps = kpps.tile([GH, NB], F32, name=f"kq2_ps_{j}", tag="kp")
                nc.tensor.matmul(kq2_ps, pswap_sb, kpr_sb, start=True, stop=True)
                kq2_sb = krope.tile([GH, NB], QK_DT, name=f"kq2_sb_{j}", tag="kq2")
                nc.scalar.copy(kq2_sb, kq2_ps)

                # ---- Q path + scores + softmax, streamed over s ----
                for it in range(NT):
                    s0 = it * ST
                    qi = s0 // SQ
                    sq0 = s0 - qi * SQ

                    q_sb = qin.tile([D, G, ST], QPROJ_DT, name=f"q_{j}_{it}", tag="q")
                    nc.sync.dma_start(
                        out=q_sb,
                        in_=qT[j, :, :, s0:s0 + ST].rearrange("c p s -> p c s"),
                    )
                    qp_ps = qps.tile([GH, ST], F32, name=f"qp_{j}_{it}", tag="qp")
                    for c in range(G):
                        nc.tensor.matmul(
                            qp_ps, wq_sb[j, c], q_sb[:, c, :],
                            start=(c == 0), stop=(c == G - 1),
                        )
                    qp_sb = qpsb.tile([GH, ST], QK_DT, name=f"qpsb_{j}_{it}", tag="qpsb")
                    nc.scalar.copy(qp_sb, qp_ps)
                    a_sb = absb.tile([GH, ST], QK_DT, name=f"a_{j}_{it}", tag="a")
                    nc.vector.tensor_mul(a_sb, qp_sb, cos_sb[qi][:, sq0:sq0 + ST])
                    b_sb = absb.tile([GH, ST], QK_DT, name=f"b_{j}_{it}", tag="b")
                    nc.vector.tensor_mul(b_sb, qp_sb, tq_sb[qi][:, sq0:sq0 + ST])

                    at_ps = atps.tile([128, ST], F32, name=f"at_{j}_{it}", tag="at")
                    for cc in range(CPT):
                        col = slice(cc * 128, cc * 128 + 128)
                        nc.tensor.matmul(
                            at_ps[:, col], a_sb[:, col], kpr_sb,
                            start=True, stop=False,
                        )
                        nc.tensor.matmul(
                            at_ps[:, col], b_sb[:, col], kq2_sb,
                            start=False, stop=False,
                        )
                        gc = it * CPT + cc
                        nc.tensor.matmul(
                            at_ps[:, col], ident_sb,
                            mask_sb[qi][:, gc - qi * (SQ // 128), :],
                            start=False, stop=True,
                        )
                    e_sb = esb.tile([128, ST], F32, name=f"e_{j}_{it}", tag="e")
                    nc.scalar.activation(e_sb, at_ps, mybir.ActivationFunctionType.Exp)
                    sums = ssb.tile([128, CPT], F32, name=f"sums_{j}_{it}", tag="sums")
                    nc.vector.reduce_sum(
                        out=sums,
                        in_=e_sb.rearrange("p (c n) -> p c n", n=NB),
                        axis=mybir.AxisListType.X,
                    )
                    rec = ssb.tile([128, CPT], F32, name=f"rec_{j}_{it}", tag="rec")
                    nc.vector.reciprocal(rec, sums)
                    o_sb = osb.tile([128, CPT, NB], F32, name=f"o_{j}_{it}", tag="o")
                    for cc in range(CPT):
                        col = slice(cc * 128, cc * 128 + 128)
                        if NORM_ON_ACT:
                            nc.scalar.mul(
                                o_sb[:, cc, :], e_sb[:, col], rec[:, cc:cc + 1]
                            )
                        else:
                            nc.vector.tensor_scalar_mul(
                                o_sb[:, cc, :], e_sb[:, col], rec[:, cc:cc + 1]
                            )
                    nc.sync.dma_start(
                        out=out[j, s0:s0 + ST, :].rearrange("(c p) n -> p c n", p=128),
                        in_=o_sb,
                    )
    nc.compile()
    return nc


_BUILT = None


def _get_built():
    global _BUILT
    if _BUILT is None:
        _BUILT = build_bass()
    return _BUILT


def prep_inputs(q, k, attention_mask, cos_q, sin_q, cos_k, sin_k, wq, wk):
    """Slice + lay out the full inputs into 8 per-core input maps (numpy)."""
    q = np.asarray(q, dtype=np.float32)
    k = np.asarray(k, dtype=np.float32)
    attention_mask = np.asarray(attention_mask, dtype=np.float32)
    cos_q = np.asarray(cos_q, dtype=np.float32)
    sin_q = np.asarray(sin_q, dtype=np.float32)
    cos_k = np.asarray(cos_k, dtype=np.float32)
    sin_k = np.asarray(sin_k, dtype=np.float32)
    wq = np.asarray(wq, dtype=np.float32)
    wk = np.asarray(wk, dtype=np.float32)

    scale = np.float32(1.0 / math.sqrt(GH))
    wq_s = (wq * scale).astype(np.float32)                  # fold score scale
    wk_f = wk.reshape(HK, 2, D, GH).copy()
    wk_f[:, 0, :, :] *= np.float32(1.0 / BLK)               # fold mean pooling

    ident = np.eye(128, dtype=_np_dt(BF16))
    pswap = np.zeros((128, 128), dtype=_np_dt(QK_DT))
    pswap[(np.arange(128) + H) % 128, np.arange(128)] = 1.0

    qproj_dt = _np_dt(QPROJ_DT)
    cos_dt = _np_dt(COS_DT)
    kt_dt = _np_dt(KT_DT)
    in_maps = []
    for core in range(N_CORES):
        b = core // 4
        kh0 = 2 * (core % 4)
        qs = q[b, :, 4 * kh0:4 * kh0 + 8, :]                # [S, 8, D]
        qTm = np.ascontiguousarray(
            qs.reshape(S, 2, G, D).transpose(1, 2, 3, 0)    # [2, G, D, S]
        ).astype(qproj_dt)
        kTm = np.ascontiguousarray(
            k[b, :, kh0:kh0 + 2, :].transpose(1, 2, 0)      # [2, D, S]
        ).astype(kt_dt)
        cq = cos_q[b]                                       # [S, GH]
        sq = sin_q[b]
        # tq[h, s] = sin[s, h+64] for h<64 ; -sin[s, h-64] for h>=64
        tqm = np.concatenate([sq[:, H:], -sq[:, :H]], axis=1).T
        ck = cos_k[b]                                       # [NB, GH]
        sk = sin_k[b]
        sks = np.concatenate([-sk[:, :H], sk[:, H:]], axis=1).T
        in_maps.append({
            "qT": qTm,
            "kT": kTm,
            "wq": np.ascontiguousarray(
                wq_s[kh0:kh0 + 2].reshape(2, G, D, GH)
            ).astype(qproj_dt),
            "wk": np.ascontiguousarray(wk_f[kh0:kh0 + 2]),
            "maskp": attention_mask[b, 0].astype(_np_dt(BF16)),
            "cosq": np.ascontiguousarray(cq.T).astype(cos_dt),
            "tq": np.ascontiguousarray(tqm).astype(cos_dt),
            "cosk": np.ascontiguousarray(ck.T),
            "sinks": np.ascontiguousarray(sks),
            "ident": ident,
            "pswap": pswap,
        })
    return in_maps


def run_cores(in_maps, **kwargs):
    nc = _get_built()
    return run_bass_kernel_spmd(nc, in_maps, core_ids=list(range(N_CORES)), **kwargs)


def kernel(**inputs):
    in_maps = prep_inputs(**inputs)
    res = run_cores(in_maps)
    full = np.empty((B, HK, S, NB), dtype=np.float32)
    for core in range(N_CORES):
        b = core // 4
        kh0 = 2 * (core % 4)
        o = np.asarray(res.results[core]["out"], dtype=np.float32)
        full[b, kh0] = o[0]
        full[b, kh0 + 1] = o[1]
    return full


# revision 9
# speedup vs baseline: 1.6152x; 1.0377x over previous
"""Trainium2 Bass kernel for nn_AttnGate (block-sparse attention gate).

Computation (per batch b, kv-head kh):
    qp  = einsum('s(gd),(gd)o->so', q[b,:,4kh:4kh+4,:], wq[kh])       # [S, GH]
    qpR = rope(qp, cos_q[b], sin_q[b])
    kc  = [mean, max] pooling of k[b,:,kh,:] over 64-key blocks        # [NB, 2D]
    kp  = kc @ wk[kh];  kpR = rope(kp, cos_k[b], sin_k[b])             # [NB, GH]
    out = softmax(qpR @ kpR.T / sqrt(GH) + mask[b])                    # [S, NB]

Sharding: 16 (b, kh) units over 8 cores; core c handles b = c//4 and
kh in {2*(c%4), 2*(c%4)+1}.  No collectives; host slices/gathers.

Host-side layout prep (inside kernel(), numpy only):
  - q/k slices pre-transposed to contraction-major ([gd, S] / [d, S]) so no
    on-chip fp32 transposes are needed; q-path and score operands pre-cast to
    fp16 (|values| ~ O(1), well inside fp16 range).
  - 1/sqrt(GH) folded into wq; 1/64 mean-pool scale folded into wk[:, :D].
  - RoPE rotate-half is folded into the QK matmul:
        attn[s,n] = sum_h qp[s,h]*cos[s,h]*kpR[n,h]
                  + sum_h qp[s,h]*tq[s,h]*kq2[n,h]
    with tq = swap_halves(sin_q) (upper half negated) and
    kq2 = swap_halves(kpR) (computed on-chip via a permutation matmul), so
    the per-s-row work is just two elementwise multiplies.
  - mask is pre-cast to bf16 (exact for 0/-1e9) and added into PSUM with an
    identity-matrix matmul.
  - block pooling is a pairwise fp16 tensor-tensor tree (2x DVE mode) rather
    than a 1x tensor_reduce.
"""

import math
import os
import sys

import numpy as np

for _p in ("/opt/trn_rl_repo", "/root/.axon_site/_ro/trn_rl_repo"):
    if os.path.isdir(_p) and _p not in sys.path:
        sys.path.append(_p)

import ml_dtypes  # noqa: E402,F401

import concourse.bass as bass  # noqa: E402,F401
import concourse.bacc as bacc  # noqa: E402
import concourse.mybir as mybir  # noqa: E402
from concourse.bass_utils import run_bass_kernel_spmd  # noqa: E402
from concourse.tile import TileContext  # noqa: E402

# Problem dims (hardcoded per spec).
B, S, HQ, HK, D, GH = 2, 8192, 32, 8, 128, 128
BLK = 64
NB = S // BLK          # 128 key blocks
G = HQ // HK           # 4
GD = G * D             # 512 contraction for the q projection
H = GH // 2            # rotate-half split
ST = 512               # s-tile (matmul moving-dim)
NT = S // ST           # 16 s-tiles per unit
CPT = ST // 128        # 128-row chunks per s-tile (4)
N_CORES = 8

F32 = mybir.dt.float32
F32R = mybir.dt.float32r
BF16 = mybir.dt.bfloat16
FP16 = mybir.dt.float16
FP = mybir.dt.np  # dt -> numpy dtype

UNITS = int(os.environ.get("KERNEL_UNITS", "2"))

# Dtype knobs (precision/perf tradeoffs).
QPROJ_DT = FP16       # q, wq and the q-projection matmul
QK_DT = FP16          # qp/a/bq/kpr/kq2 and the score matmuls
COS_DT = FP16         # resident cos/tq tables
KT_DT = FP16          # k in [d, s] layout + pooling tree
NORM_ON_ACT = True    # softmax normalize multiply on ScalarE (else VectorE)


def _np_dt(dt):
    return np.dtype(FP(dt))


def build_bass():
    """Build the single-core SPMD Bass program (same NEFF on all 8 cores)."""
    nc = bacc.Bacc("TRN2")

    qT = nc.declare_dram_parameter("qT", [2, G, D, S], QPROJ_DT, isOutput=False)
    kT = nc.declare_dram_parameter("kT", [2, D, S], KT_DT, isOutput=False)
    wq = nc.declare_dram_parameter("wq", [2, G, D, GH], QPROJ_DT, isOutput=False)
    wk = nc.declare_dram_parameter("wk", [2, 2, D, GH], F32, isOutput=False)
    maskp = nc.declare_dram_parameter("maskp", [S, NB], BF16, isOutput=False)
    cosq = nc.declare_dram_parameter("cosq", [GH, S], COS_DT, isOutput=False)
    tq = nc.declare_dram_parameter("tq", [GH, S], COS_DT, isOutput=False)
    cosk = nc.declare_dram_parameter("cosk", [GH, NB], F32, isOutput=False)
    sinks = nc.declare_dram_parameter("sinks", [GH, NB], F32, isOutput=False)
    ident = nc.declare_dram_parameter("ident", [128, 128], BF16, isOutput=False)
    pswap = nc.declare_dram_parameter("pswap", [128, 128], QK_DT, isOutput=False)
    out = nc.declare_dram_parameter("out", [2, S, NB], F32, isOutput=True)

    NQUART = 4
    SQ = S // NQUART  # 2048 cols per resident-table quarter

    with TileContext(nc) as tc:
        with (
            tc.tile_pool(name="singles", bufs=1) as singles,
            tc.tile_pool(name="ktp", bufs=2) as ktp,
            tc.tile_pool(name="pooltmp", bufs=1) as pooltmp,
            tc.tile_pool(name="kstat", bufs=2) as kstat,
            tc.tile_pool(name="krope", bufs=2) as krope,
            tc.tile_pool(name="qin", bufs=6) as qin,
            tc.tile_pool(name="qps", bufs=2, space="PSUM") as qps,
            tc.tile_pool(name="kpps", bufs=2, space="PSUM") as kpps,
            tc.tile_pool(name="atps", bufs=3, space="PSUM") as atps,
            tc.tile_pool(name="qpsb", bufs=3) as qpsb,
            tc.tile_pool(name="absb", bufs=3) as absb,
            tc.tile_pool(name="esb", bufs=3) as esb,
            tc.tile_pool(name="ssb", bufs=3) as ssb,
            tc.tile_pool(name="osb", bufs=3) as osb,
        ):
            # ---- resident constants ----
            wq_sb = {}
            for j in range(UNITS):
                for c in range(G):
                    t = singles.tile(
                        [D, GH], QPROJ_DT, name=f"wq_{j}_{c}", tag=f"wq{j}{c}"
                    )
                    nc.sync.dma_start(out=t, in_=wq[j, c])
                    wq_sb[j, c] = t
            wk_sb = {}
            for j in range(UNITS):
                for c in range(2):
                    t = singles.tile([D, GH], F32, name=f"wk_{j}_{c}", tag=f"wk{j}{c}")
                    nc.sync.dma_start(out=t, in_=wk[j, c])
                    wk_sb[j, c] = t
            cosk_sb = singles.tile([GH, NB], F32, name="cosk_sb", tag="cosk")
            nc.sync.dma_start(out=cosk_sb, in_=cosk[:, :])
            sinks_sb = singles.tile([GH, NB], F32, name="sinks_sb", tag="sinks")
            nc.sync.dma_start(out=sinks_sb, in_=sinks[:, :])
            ident_sb = singles.tile([128, 128], BF16, name="ident_sb", tag="ident")
            nc.sync.dma_start(out=ident_sb, in_=ident[:, :])
            pswap_sb = singles.tile([128, 128], QK_DT, name="pswap_sb", tag="pswap")
            nc.sync.dma_start(out=pswap_sb, in_=pswap[:, :])

            # Resident tables, split into quarters, loaded lazily (first use
            # order == trace order == scheduler priority) to shorten startup.
            cos_sb, tq_sb, mask_sb = {}, {}, {}

            def load_quarter(qi):
                if qi in cos_sb:
                    return
                t = singles.tile([GH, SQ], COS_DT, name=f"cos_sb_{qi}", tag=f"cosq{qi}")
                nc.sync.dma_start(out=t, in_=cosq[:, qi * SQ:(qi + 1) * SQ])
                cos_sb[qi] = t
                t = singles.tile([GH, SQ], COS_DT, name=f"tq_sb_{qi}", tag=f"tqq{qi}")
                nc.sync.dma_start(out=t, in_=tq[:, qi * SQ:(qi + 1) * SQ])
                tq_sb[qi] = t
                t = singles.tile(
                    [128, SQ // 128, NB], BF16, name=f"mask_sb_{qi}", tag=f"mask{qi}"
                )
                nc.sync.dma_start(
                    out=t,
                    in_=maskp[qi * SQ:(qi + 1) * SQ, :].rearrange(
                        "(c p) n -> p c n", p=128
                    ),
                )
                mask_sb[qi] = t

            def pool_tree(kt_sb, op, outname):
                """Pairwise-[op] over the 64-key blocks of kt_sb [D, S] ->
                [D, NB] f32, via fp16 2x tensor_tensor levels."""
                cur = kt_sb.rearrange("p (n b) -> p n b", b=BLK)
                width = BLK
                while width > 2:
                    width //= 2
                    t = pooltmp.tile(
                        [D, NB, width], KT_DT,
                        name=f"{outname}_l{width}", tag=f"pool{width}",
                    )
                    nc.vector.tensor_tensor(
                        out=t, in0=cur[:, :, :width], in1=cur[:, :, width:], op=op
                    )
                    cur = t
                res = kstat.tile([D, NB], F32, name=outname, tag=outname[:4])
                nc.vector.tensor_tensor(
                    out=res, in0=cur[:, :, 0], in1=cur[:, :, 1], op=op
                )
                return res

            for j in range(UNITS):  # the (b, kh) units on this core
                # ---- K path ----
                kt_sb = ktp.tile([D, S], KT_DT, name=f"kt_{j}", tag="kt")
                nc.sync.dma_start(out=kt_sb, in_=kT[j])
                ksum = pool_tree(kt_sb, mybir.AluOpType.add, f"ksum_{j}")
                kmax = pool_tree(kt_sb, mybir.AluOpType.max, f"kmax_{j}")

                kp_ps = kpps.tile([GH, NB], F32, name=f"kp_ps_{j}", tag="kp")
                nc.tensor.matmul(kp_ps, wk_sb[j, 0], ksum, start=True, stop=False)
                nc.tensor.matmul(kp_ps, wk_sb[j, 1], kmax, start=False, stop=True)
                kp_sb = krope.tile([GH, NB], QK_DT, name=f"kp_sb_{j}", tag="kpsb")
                nc.scalar.copy(kp_sb, kp_ps)
                # kp2 = swap_halves(kp) via permutation matmul
                kp2_ps = kpps.tile([GH, NB], F32, name=f"kp2_ps_{j}", tag="kp")
                nc.tensor.matmul(kp2_ps, pswap_sb, kp_sb, start=True, stop=True)
                # kpR = kp*cosk + kp2*sinks  (sign folded into sinks on host)
                u_sb = krope.tile([GH, NB], F32, name=f"u_sb_{j}", tag="usb")
                nc.vector.tensor_mul(u_sb, kp_ps, cosk_sb)
                v_sb = krope.tile([GH, NB], F32, name=f"v_sb_{j}", tag="vsb")
                nc.vector.tensor_mul(v_sb, kp2_ps, sinks_sb)
                kpr_sb = krope.tile([GH, NB], QK_DT, name=f"kpr_sb_{j}", tag="kpr")
                nc.vector.tensor_add(kpr_sb, u_sb, v_sb)
                # kq2 = swap_halves(kpR) via permutation matmul
                kq2_ps = kpps.tile([GH, NB], F32, name=f"kq2_ps_{j}", tag="kp")
                nc.tensor.matmul(kq2_ps, pswap_sb, kpr_sb, start=True, stop=True)
                kq2_sb = krope.tile([GH, NB], QK_DT, name=f"kq2_sb_{j}", tag="kq2")
                nc.scalar.copy(kq2_sb, kq2_ps)

                # ---- Q path + scores + softmax, streamed over s ----
                for it in range(NT):
                    s0 = it * ST
                    qi = s0 // SQ
                    sq0 = s0 - qi * SQ
                    load_quarter(qi)

                    q_sb = qin.tile([D, G, ST], QPROJ_DT, name=f"q_{j}_{it}", tag="q")
                    nc.sync.dma_start(
                        out=q_sb,
                        in_=qT[j, :, :, s0:s0 + ST].rearrange("c p s -> p c s"),
                    )
                    qp_ps = qps.tile([GH, ST], F32, name=f"qp_{j}_{it}", tag="qp")
                    for c in range(G):
                        nc.tensor.matmul(
                            qp_ps, wq_sb[j, c], q_sb[:, c, :],
                            start=(c == 0), stop=(c == G - 1),
                        )
                    qp_sb = qpsb.tile([GH, ST], QK_DT, name=f"qpsb_{j}_{it}", tag="qpsb")
                    nc.scalar.copy(qp_sb, qp_ps)
                    a_sb = absb.tile([GH, ST], QK_DT, name=f"a_{j}_{it}", tag="a")
                    nc.vector.tensor_mul(a_sb, qp_sb, cos_sb[qi][:, sq0:sq0 + ST])
                    b_sb = absb.tile([GH, ST], QK_DT, name=f"b_{j}_{it}", tag="b")
                    nc.vector.tensor_mul(b_sb, qp_sb, tq_sb[qi][:, sq0:sq0 + ST])

                    at_ps = atps.tile([128, ST], F32, name=f"at_{j}_{it}", tag="at")
                    for cc in range(CPT):
                        col = slice(cc * 128, cc * 128 + 128)
                        nc.tensor.matmul(
                            at_ps[:, col], a_sb[:, col], kpr_sb,
                            start=True, stop=False,
                        )
                        nc.tensor.matmul(
                            at_ps[:, col], b_sb[:, col], kq2_sb,
                            start=False, stop=False,
                        )
                        gc = it * CPT + cc
                        nc.tensor.matmul(
                            at_ps[:, col], ident_sb,
                            mask_sb[qi][:, gc - qi * (SQ // 128), :],
                            start=False, stop=True,
                        )
                    e_sb = esb.tile([128, ST], F32, name=f"e_{j}_{it}", tag="e")
                    nc.scalar.activation(e_sb, at_ps, mybir.ActivationFunctionType.Exp)
                    sums = ssb.tile([128, CPT], F32, name=f"sums_{j}_{it}", tag="sums")
                    nc.vector.reduce_sum(
                        out=sums,
                        in_=e_sb.rearrange("p (c n) -> p c n", n=NB),
                        axis=mybir.AxisListType.X,
                    )
                    rec = ssb.tile([128, CPT], F32, name=f"rec_{j}_{it}", tag="rec")
                    nc.vector.reciprocal(rec, sums)
                    o_sb = osb.tile([128, CPT, NB], F32, name=f"o_{j}_{it}", tag="o")
                    rec_b = bass.AP(
                        tensor=rec.tensor, offset=rec.offset,
                        ap=[rec.ap[0], rec.ap[1], [0, NB]],
                    )
                    nc.vector.tensor_tensor(
                        out=o_sb, in0=e_sb.rearrange("p (c n) -> p c n", n=NB),
                        in1=rec_b, op=mybir.AluOpType.mult,
                    )
                    nc.sync.dma_start(
                        out=out[j, s0:s0 + ST, :].rearrange("(c p) n -> p c n", p=128),
                        in_=o_sb,
                    )
    nc.compile()
    return nc


_BUILT = None


def _get_built():
    global _BUILT
    if _BUILT is None:
        _BUILT = build_bass()
    return _BUILT


def prep_inputs(q, k, attention_mask, cos_q, sin_q, cos_k, sin_k, wq, wk):
    """Slice + lay out the full inputs into 8 per-core input maps (numpy)."""
    q = np.asarray(q, dtype=np.float32)
    k = np.asarray(k, dtype=np.float32)
    attention_mask = np.asarray(attention_mask, dtype=np.float32)
    cos_q = np.asarray(cos_q, dtype=np.float32)
    sin_q = np.asarray(sin_q, dtype=np.float32)
    cos_k = np.asarray(cos_k, dtype=np.float32)
    sin_k = np.asarray(sin_k, dtype=np.float32)
    wq = np.asarray(wq, dtype=np.float32)
    wk = np.asarray(wk, dtype=np.float32)

    scale = np.float32(1.0 / math.sqrt(GH))
    wq_s = (wq * scale).astype(np.float32)                  # fold score scale
    wk_f = wk.reshape(HK, 2, D, GH).copy()
    wk_f[:, 0, :, :] *= np.float32(1.0 / BLK)               # fold mean pooling

    ident = np.eye(128, dtype=_np_dt(BF16))
    pswap = np.zeros((128, 128), dtype=_np_dt(QK_DT))
    pswap[(np.arange(128) + H) % 128, np.arange(128)] = 1.0

    qproj_dt = _np_dt(QPROJ_DT)
    cos_dt = _np_dt(COS_DT)
    kt_dt = _np_dt(KT_DT)
    in_maps = []
    for core in range(N_CORES):
        b = core // 4
        kh0 = 2 * (core % 4)
        qs = q[b, :, 4 * kh0:4 * kh0 + 8, :]                # [S, 8, D]
        qTm = np.ascontiguousarray(
            qs.reshape(S, 2, G, D).transpose(1, 2, 3, 0)    # [2, G, D, S]
        ).astype(qproj_dt)
        kTm = np.ascontiguousarray(
            k[b, :, kh0:kh0 + 2, :].transpose(1, 2, 0)      # [2, D, S]
        ).astype(kt_dt)
        cq = cos_q[b]                                       # [S, GH]
        sq = sin_q[b]
        # tq[h, s] = sin[s, h+64] for h<64 ; -sin[s, h-64] for h>=64
        tqm = np.concatenate([sq[:, H:], -sq[:, :H]], axis=1).T
        ck = cos_k[b]                                       # [NB, GH]
        sk = sin_k[b]
        sks = np.concatenate([-sk[:, :H], sk[:, H:]], axis=1).T
        in_maps.append({
            "qT": qTm,
            "kT": kTm,
            "wq": np.ascontiguousarray(
                wq_s[kh0:kh0 + 2].reshape(2, G, D, GH)
            ).astype(qproj_dt),
            "wk": np.ascontiguousarray(wk_f[kh0:kh0 + 2]),
            "maskp": attention_mask[b, 0].astype(_np_dt(BF16)),
            "cosq": np.ascontiguousarray(cq.T).astype(cos_dt),
            "tq": np.ascontiguousarray(tqm).astype(cos_dt),
            "cosk": np.ascontiguousarray(ck.T),
            "sinks": np.ascontiguousarray(sks),
            "ident": ident,
            "pswap": pswap,
        })
    return in_maps


def run_cores(in_maps, **kwargs):
    nc = _get_built()
    return run_bass_kernel_spmd(nc, in_maps, core_ids=list(range(N_CORES)), **kwargs)


def kernel(**inputs):
    in_maps = prep_inputs(**inputs)
    res = run_cores(in_maps)
    full = np.empty((B, HK, S, NB), dtype=np.float32)
    for core in range(N_CORES):
        b = core // 4
        kh0 = 2 * (core % 4)
        o = np.asarray(res.results[core]["out"], dtype=np.float32)
        full[b, kh0] = o[0]
        full[b, kh0 + 1] = o[1]
    return full


# revision 10
# speedup vs baseline: 1.7311x; 1.0717x over previous
"""Trainium2 Bass kernel for nn_AttnGate (block-sparse attention gate).

Computation (per batch b, kv-head kh):
    qp  = einsum('s(gd),(gd)o->so', q[b,:,4kh:4kh+4,:], wq[kh])       # [S, GH]
    qpR = rope(qp, cos_q[b], sin_q[b])
    kc  = [mean, max] pooling of k[b,:,kh,:] over 64-key blocks        # [NB, 2D]
    kp  = kc @ wk[kh];  kpR = rope(kp, cos_k[b], sin_k[b])             # [NB, GH]
    out = softmax(qpR @ kpR.T / sqrt(GH) + mask[b])                    # [S, NB]

Sharding: 16 (b, kh) units over 8 cores; core c handles b = c//4 and
kh in {2*(c%4), 2*(c%4)+1}.  No collectives; host slices/gathers.

Host-side layout prep (inside kernel(), numpy only):
  - q/k slices pre-transposed to contraction-major ([gd, S] / [d, S]) so no
    on-chip fp32 transposes are needed; q-path and score operands pre-cast to
    fp16 (|values| ~ O(1), well inside fp16 range).
  - 1/sqrt(GH) folded into wq; 1/64 mean-pool scale folded into wk[:, :D].
  - RoPE rotate-half is folded into the QK matmul:
        attn[s,n] = sum_h qp[s,h]*cos[s,h]*kpR[n,h]
                  + sum_h qp[s,h]*tq[s,h]*kq2[n,h]
    with tq = swap_halves(sin_q) (upper half negated) and
    kq2 = swap_halves(kpR) (computed on-chip via a permutation matmul), so
    the per-s-row work is just two elementwise multiplies.
  - mask is pre-cast to bf16 (exact for 0/-1e9) and added into PSUM with an
    identity-matrix matmul.
  - block pooling is a pairwise fp16 tensor-tensor tree (2x DVE mode) rather
    than a 1x tensor_reduce.
"""

import math
import os
import sys

import numpy as np

for _p in ("/opt/trn_rl_repo", "/root/.axon_site/_ro/trn_rl_repo"):
    if os.path.isdir(_p) and _p not in sys.path:
        sys.path.append(_p)

import ml_dtypes  # noqa: E402,F401

import concourse.bass as bass  # noqa: E402,F401
import concourse.bacc as bacc  # noqa: E402
import concourse.mybir as mybir  # noqa: E402
from concourse.bass_utils import run_bass_kernel_spmd  # noqa: E402
from concourse.tile import TileContext  # noqa: E402

# Problem dims (hardcoded per spec).
B, S, HQ, HK, D, GH = 2, 8192, 32, 8, 128, 128
BLK = 64
NB = S // BLK          # 128 key blocks
G = HQ // HK           # 4
GD = G * D             # 512 contraction for the q projection
H = GH // 2            # rotate-half split
ST = 512               # s-tile (matmul moving-dim)
NT = S // ST           # 16 s-tiles per unit
CPT = ST // 128        # 128-row chunks per s-tile (4)
N_CORES = 8

F32 = mybir.dt.float32
F32R = mybir.dt.float32r
BF16 = mybir.dt.bfloat16
FP16 = mybir.dt.float16
FP = mybir.dt.np  # dt -> numpy dtype

UNITS = int(os.environ.get("KERNEL_UNITS", "2"))

# Dtype knobs (precision/perf tradeoffs).
QPROJ_DT = FP16       # q, wq and the q-projection matmul
QK_DT = FP16          # qp/a/bq/kpr/kq2 and the score matmuls
COS_DT = FP16         # resident cos/tq tables
KT_DT = FP16          # k in [d, s] layout + pooling tree
OUT_DT = BF16         # output written bf16 (exact exponent range), host upcasts


def _np_dt(dt):
    return np.dtype(FP(dt))


def build_bass():
    """Build the single-core SPMD Bass program (same NEFF on all 8 cores)."""
    nc = bacc.Bacc("TRN2")

    qT = nc.declare_dram_parameter("qT", [2, G, D, S], QPROJ_DT, isOutput=False)
    kT = nc.declare_dram_parameter("kT", [2, D, S], KT_DT, isOutput=False)
    wq = nc.declare_dram_parameter("wq", [2, G, D, GH], QPROJ_DT, isOutput=False)
    wk = nc.declare_dram_parameter("wk", [2, 2, D, GH], F32, isOutput=False)
    maskp = nc.declare_dram_parameter("maskp", [S, NB], BF16, isOutput=False)
    ct = nc.declare_dram_parameter("ct", [GH, 2, S], COS_DT, isOutput=False)
    cosk = nc.declare_dram_parameter("cosk", [GH, NB], F32, isOutput=False)
    sinks = nc.declare_dram_parameter("sinks", [GH, NB], F32, isOutput=False)
    ident = nc.declare_dram_parameter("ident", [128, 128], BF16, isOutput=False)
    pswap = nc.declare_dram_parameter("pswap", [128, 128], QK_DT, isOutput=False)
    out = nc.declare_dram_parameter("out", [2, S, NB], OUT_DT, isOutput=True)

    NQUART = 4
    SQ = S // NQUART  # 2048 cols per resident-table quarter

    with TileContext(nc) as tc:
        with (
            tc.tile_pool(name="singles", bufs=1) as singles,
            tc.tile_pool(name="ktp", bufs=2) as ktp,
            tc.tile_pool(name="pooltmp", bufs=1) as pooltmp,
            tc.tile_pool(name="kstat", bufs=2) as kstat,
            tc.tile_pool(name="krope", bufs=2) as krope,
            tc.tile_pool(name="qin", bufs=6) as qin,
            tc.tile_pool(name="qps", bufs=2, space="PSUM") as qps,
            tc.tile_pool(name="kpps", bufs=2, space="PSUM") as kpps,
            tc.tile_pool(name="atps", bufs=3, space="PSUM") as atps,
            tc.tile_pool(name="qpsb", bufs=3) as qpsb,
            tc.tile_pool(name="absb", bufs=3) as absb,
            tc.tile_pool(name="esb", bufs=3) as esb,
            tc.tile_pool(name="ssb", bufs=3) as ssb,
            tc.tile_pool(name="osb", bufs=3) as osb,
        ):
            # ---- resident constants ----
            wq_sb = {}
            for j in range(UNITS):
                for c in range(G):
                    t = singles.tile(
                        [D, GH], QPROJ_DT, name=f"wq_{j}_{c}", tag=f"wq{j}{c}"
                    )
                    nc.sync.dma_start(out=t, in_=wq[j, c])
                    wq_sb[j, c] = t
            wk_sb = {}
            for j in range(UNITS):
                for c in range(2):
                    t = singles.tile([D, GH], F32, name=f"wk_{j}_{c}", tag=f"wk{j}{c}")
                    nc.sync.dma_start(out=t, in_=wk[j, c])
                    wk_sb[j, c] = t
            cosk_sb = singles.tile([GH, NB], F32, name="cosk_sb", tag="cosk")
            nc.sync.dma_start(out=cosk_sb, in_=cosk[:, :])
            sinks_sb = singles.tile([GH, NB], F32, name="sinks_sb", tag="sinks")
            nc.sync.dma_start(out=sinks_sb, in_=sinks[:, :])
            ident_sb = singles.tile([128, 128], BF16, name="ident_sb", tag="ident")
            nc.sync.dma_start(out=ident_sb, in_=ident[:, :])
            pswap_sb = singles.tile([128, 128], QK_DT, name="pswap_sb", tag="pswap")
            nc.sync.dma_start(out=pswap_sb, in_=pswap[:, :])

            # Resident tables, split into quarters, loaded lazily (first use
            # order == trace order == scheduler priority) to shorten startup.
            ct_sb, mask_sb = {}, {}

            def load_quarter(qi):
                if qi in ct_sb:
                    return
                t = singles.tile([GH, 2, SQ], COS_DT, name=f"ct_sb_{qi}", tag=f"ctq{qi}")
                nc.sync.dma_start(out=t, in_=ct[:, :, qi * SQ:(qi + 1) * SQ])
                ct_sb[qi] = t
                t = singles.tile(
                    [128, SQ // 128, NB], BF16, name=f"mask_sb_{qi}", tag=f"mask{qi}"
                )
                nc.sync.dma_start(
                    out=t,
                    in_=maskp[qi * SQ:(qi + 1) * SQ, :].rearrange(
                        "(c p) n -> p c n", p=128
                    ),
                )
                mask_sb[qi] = t

            def pool_tree(kt_sb, op, outname):
                """Pairwise-[op] over the 64-key blocks of kt_sb [D, S] ->
                [D, NB] f32, via fp16 2x tensor_tensor levels."""
                cur = kt_sb.rearrange("p (n b) -> p n b", b=BLK)
                width = BLK
                while width > 2:
                    width //= 2
                    t = pooltmp.tile(
                        [D, NB, width], KT_DT,
                        name=f"{outname}_l{width}", tag=f"pool{width}",
                    )
                    nc.vector.tensor_tensor(
                        out=t, in0=cur[:, :, :width], in1=cur[:, :, width:], op=op
                    )
                    cur = t
                res = kstat.tile([D, NB], F32, name=outname, tag=outname[:4])
                nc.vector.tensor_tensor(
                    out=res, in0=cur[:, :, 0], in1=cur[:, :, 1], op=op
                )
                return res

            for j in range(UNITS):  # the (b, kh) units on this core
                # ---- K path ----
                kt_sb = ktp.tile([D, S], KT_DT, name=f"kt_{j}", tag="kt")
                nc.sync.dma_start(out=kt_sb, in_=kT[j])
                ksum = pool_tree(kt_sb, mybir.AluOpType.add, f"ksum_{j}")
                kmax = pool_tree(kt_sb, mybir.AluOpType.max, f"kmax_{j}")

                kp_ps = kpps.tile([GH, NB], F32, name=f"kp_ps_{j}", tag="kp")
                nc.tensor.matmul(kp_ps, wk_sb[j, 0], ksum, start=True, stop=False)
                nc.tensor.matmul(kp_ps, wk_sb[j, 1], kmax, start=False, stop=True)
                kp_sb = krope.tile([GH, NB], QK_DT, name=f"kp_sb_{j}", tag="kpsb")
                nc.scalar.copy(kp_sb, kp_ps)
                # kp2 = swap_halves(kp) via permutation matmul
                kp2_ps = kpps.tile([GH, NB], F32, name=f"kp2_ps_{j}", tag="kp")
                nc.tensor.matmul(kp2_ps, pswap_sb, kp_sb, start=True, stop=True)
                # kpR = kp*cosk + kp2*sinks  (sign folded into sinks on host)
                u_sb = krope.tile([GH, NB], F32, name=f"u_sb_{j}", tag="usb")
                nc.vector.tensor_mul(u_sb, kp_ps, cosk_sb)
                v_sb = krope.tile([GH, NB], F32, name=f"v_sb_{j}", tag="vsb")
                nc.vector.tensor_mul(v_sb, kp2_ps, sinks_sb)
                kpr_sb = krope.tile([GH, NB], QK_DT, name=f"kpr_sb_{j}", tag="kpr")
                nc.vector.tensor_add(kpr_sb, u_sb, v_sb)
                # kq2 = swap_halves(kpR) via permutation matmul
                kq2_ps = kpps.tile([GH, NB], F32, name=f"kq2_ps_{j}", tag="kp")
                nc.tensor.matmul(kq2_ps, pswap_sb, kpr_sb, start=True, stop=True)
                kq2_sb = krope.tile([GH, NB], QK_DT, name=f"kq2_sb_{j}", tag="kq2")
                nc.scalar.copy(kq2_sb, kq2_ps)

                # ---- Q path + scores + softmax, streamed over s ----
                for it in range(NT):
                    s0 = it * ST
                    qi = s0 // SQ
                    sq0 = s0 - qi * SQ
                    load_quarter(qi)

                    q_sb = qin.tile([D, G, ST], QPROJ_DT, name=f"q_{j}_{it}", tag="q")
                    nc.sync.dma_start(
                        out=q_sb,
                        in_=qT[j, :, :, s0:s0 + ST].rearrange("c p s -> p c s"),
                    )
                    qp_ps = qps.tile([GH, ST], F32, name=f"qp_{j}_{it}", tag="qp")
                    for c in range(G):
                        nc.tensor.matmul(
                            qp_ps, wq_sb[j, c], q_sb[:, c, :],
                            start=(c == 0), stop=(c == G - 1),
                        )
                    qp_sb = qpsb.tile([GH, ST], QK_DT, name=f"qpsb_{j}_{it}", tag="qpsb")
                    nc.scalar.copy(qp_sb, qp_ps)
                    ab_sb = absb.tile([GH, 2, ST], QK_DT, name=f"ab_{j}_{it}", tag="ab")
                    qp_b = bass.AP(
                        tensor=qp_sb.tensor, offset=qp_sb.offset,
                        ap=[qp_sb.ap[0], [0, 2], qp_sb.ap[1]],
                    )
                    nc.vector.tensor_tensor(
                        out=ab_sb, in0=qp_b,
                        in1=ct_sb[qi][:, :, sq0:sq0 + ST],
                        op=mybir.AluOpType.mult,
                    )
                    a_sb = ab_sb[:, 0, :]
                    b_sb = ab_sb[:, 1, :]

                    at_ps = atps.tile([128, ST], F32, name=f"at_{j}_{it}", tag="at")
                    for cc in range(CPT):
                        col = slice(cc * 128, cc * 128 + 128)
                        nc.tensor.matmul(
                            at_ps[:, col], a_sb[:, col], kpr_sb,
                            start=True, stop=False,
                        )
                        nc.tensor.matmul(
                            at_ps[:, col], b_sb[:, col], kq2_sb,
                            start=False, stop=False,
                        )
                        gc = it * CPT + cc
                        nc.tensor.matmul(
                            at_ps[:, col], ident_sb,
                            mask_sb[qi][:, gc - qi * (SQ // 128), :],
                            start=False, stop=True,
                        )
                    e_sb = esb.tile([128, ST], F32, name=f"e_{j}_{it}", tag="e")
                    nc.scalar.activation(e_sb, at_ps, mybir.ActivationFunctionType.Exp)
                    sums = ssb.tile([128, CPT], F32, name=f"sums_{j}_{it}", tag="sums")
                    nc.vector.reduce_sum(
                        out=sums,
                        in_=e_sb.rearrange("p (c n) -> p c n", n=NB),
                        axis=mybir.AxisListType.X,
                    )
                    rec = ssb.tile([128, CPT], F32, name=f"rec_{j}_{it}", tag="rec")
                    nc.vector.reciprocal(rec, sums)
                    o_sb = osb.tile([128, CPT, NB], OUT_DT, name=f"o_{j}_{it}", tag="o")
                    rec_b = bass.AP(
                        tensor=rec.tensor, offset=rec.offset,
                        ap=[rec.ap[0], rec.ap[1], [0, NB]],
                    )
                    nc.vector.tensor_tensor(
                        out=o_sb, in0=e_sb.rearrange("p (c n) -> p c n", n=NB),
                        in1=rec_b, op=mybir.AluOpType.mult,
                    )
                    nc.sync.dma_start(
                        out=out[j, s0:s0 + ST, :].rearrange("(c p) n -> p c n", p=128),
                        in_=o_sb,
                    )
    nc.compile()
    return nc


_BUILT = None


def _get_built():
    global _BUILT
    if _BUILT is None:
        _BUILT = build_bass()
    return _BUILT


def prep_inputs(q, k, attention_mask, cos_q, sin_q, cos_k, sin_k, wq, wk):
    """Slice + lay out the full inputs into 8 per-core input maps (numpy)."""
    q = np.asarray(q, dtype=np.float32)
    k = np.asarray(k, dtype=np.float32)
    attention_mask = np.asarray(attention_mask, dtype=np.float32)
    cos_q = np.asarray(cos_q, dtype=np.float32)
    sin_q = np.asarray(sin_q, dtype=np.float32)
    cos_k = np.asarray(cos_k, dtype=np.float32)
    sin_k = np.asarray(sin_k, dtype=np.float32)
    wq = np.asarray(wq, dtype=np.float32)
    wk = np.asarray(wk, dtype=np.float32)

    scale = np.float32(1.0 / math.sqrt(GH))
    wq_s = (wq * scale).astype(np.float32)                  # fold score scale
    wk_f = wk.reshape(HK, 2, D, GH).copy()
    wk_f[:, 0, :, :] *= np.float32(1.0 / BLK)               # fold mean pooling

    ident = np.eye(128, dtype=_np_dt(BF16))
    pswap = np.zeros((128, 128), dtype=_np_dt(QK_DT))
    pswap[(np.arange(128) + H) % 128, np.arange(128)] = 1.0

    qproj_dt = _np_dt(QPROJ_DT)
    cos_dt = _np_dt(COS_DT)
    kt_dt = _np_dt(KT_DT)
    in_maps = []
    for core in range(N_CORES):
        b = core // 4
        kh0 = 2 * (core % 4)
        qs = q[b, :, 4 * kh0:4 * kh0 + 8, :]                # [S, 8, D]
        qTm = np.ascontiguousarray(
            qs.reshape(S, 2, G, D).transpose(1, 2, 3, 0)    # [2, G, D, S]
        ).astype(qproj_dt)
        kTm = np.ascontiguousarray(
            k[b, :, kh0:kh0 + 2, :].transpose(1, 2, 0)      # [2, D, S]
        ).astype(kt_dt)
        cq = cos_q[b]                                       # [S, GH]
        sq = sin_q[b]
        # tq[h, s] = sin[s, h+64] for h<64 ; -sin[s, h-64] for h>=64
        tqm = np.concatenate([sq[:, H:], -sq[:, :H]], axis=1).T
        ck = cos_k[b]                                       # [NB, GH]
        sk = sin_k[b]
        sks = np.concatenate([-sk[:, :H], sk[:, H:]], axis=1).T
        ctm = np.ascontiguousarray(
            np.stack([cq.T, tqm], axis=1)                   # [GH, 2, S]
        ).astype(cos_dt)
        in_maps.append({
            "qT": qTm,
            "kT": kTm,
            "wq": np.ascontiguousarray(
                wq_s[kh0:kh0 + 2].reshape(2, G, D, GH)
            ).astype(qproj_dt),
            "wk": np.ascontiguousarray(wk_f[kh0:kh0 + 2]),
            "maskp": attention_mask[b, 0].astype(_np_dt(BF16)),
            "ct": ctm,
            "cosk": np.ascontiguousarray(ck.T),
            "sinks": np.ascontiguousarray(sks),
            "ident": ident,
            "pswap": pswap,
        })
    return in_maps


def run_cores(in_maps, **kwargs):
    nc = _get_built()
    return run_bass_kernel_spmd(nc, in_maps, core_ids=list(range(N_CORES)), **kwargs)


def kernel(**inputs):
    in_maps = prep_inputs(**inputs)
    res = run_cores(in_maps)
    full = np.empty((B, HK, S, NB), dtype=np.float32)
    for core in range(N_CORES):
        b = core // 4
        kh0 = 2 * (core % 4)
        o = np.asarray(res.results[core]["out"], dtype=np.float32)
        full[b, kh0] = o[0]
        full[b, kh0 + 1] = o[1]
    return full
